# revision 1
# baseline (speedup 1.0000x reference)
"""Trainium2 Bass kernel for nn_Attention_56487409877769.

NdLinear-qkv -> 16-head attention -> NdLinear-proj, B=4 N=1024 C=1024 H=16.

Sharding: 8 cores = (batch b, head-group g) with b=core//2, g=core%2.
Each core handles batch b and its 8 heads (qkv channel slice 512g:512g+512).
The proj channel matmul is a partial sum over the core's channel slice; the
host adds the two partials per batch plus a rank-1 bias term (the NdLinear
proj biases commute: out = Wp0 @ O @ Wp1c.T + outer(bp0, Wp1.sum(1)) + bp1).

Per-core dataflow (layouts chosen so no on-chip transpose is ever needed; all
matmul operands are float32r, which streams at full PE rate for free-dim>=256
and matches fp32 to ~1e-4):

  A : x1T[c,m] = sum_n x[b][n,c] Wq0T[n,m] + bq0[m]      (x tiles as lhsT)
  B : q, k     = Wqk1T-slices as lhsT, rhs = x1T; k kept packed [d,n], q
                 written zero-padded per head so score matmuls are K=128
                 (uniform PE tile mode - no mode-switch drains)
      v        = x1T as lhsT, rhs = Wv1T -> natural [n,d], stored in 65-wide
                 [v | ones] blocks per head
  C : ST = kT @ qpad (K=128); E = exp(SCALE*ST) on ACT (PSUM->SBUF, fp32r);
      [U;Z] = [v|ones] as lhsT, rhs=E  (U rows 0-63, Z row 64);
      normalize: recip(Z) -> 1-partition DMA hop to p0 (partition_broadcast
      ucode reads partition 0 only) -> gpsimd broadcast -> DVE multiply -> OT.
      Software-pipelined across head pairs: pair i's U-chain k-steps are
      emitted inside pair i+1's score/exp loop so the PE never idles while
      ACT streams exps (phase C is ACT-bound at ~68us).
  E'/D': Y = OT.T-contract @ Wp1cT, out = Wp0T.T-contract @ Y, interleaved by
      output half so D' starts while E''s second half still runs; E' matmuls
      overlap the last softmax tails (ep/pst pools close first).

Phase walls (TimelineSim cost model, per core): A 38 | B 43 | C 68 | E/D 46,
total ~210us vs a 164us pure-PE floor (768 MMs x 213ns).
"""

import sys

if "/opt/trn_rl_repo" not in sys.path:
    sys.path.insert(0, "/opt/trn_rl_repo")

import numpy as np

B, N, C, H = 4, 1024, 1024, 16
HD = C // H          # 64
SCALE = HD ** -0.5
P = 128
NT = N // P          # 8 partition tiles of the 1024 axes
HPC = 8              # heads per core
VW = HD + 1          # v block width per head: [ones | v]  -> 65

_CACHE = {}

LAST_RESULT = None   # test.py reads exec_time_ns / profile off this


def _build(reps=1):
    import concourse.mybir as mybir
    import concourse.tile as tile
    from concourse import bacc

    fp32 = mybir.dt.float32
    fp32r = mybir.dt.float32r
    Exp = mybir.ActivationFunctionType.Exp
    Copy = mybir.ActivationFunctionType.Copy
    Ident = mybir.ActivationFunctionType.Identity

    nc = bacc.Bacc("TRN2", target_bir_lowering=False, debug=False)

    def din(name, shape):
        return nc.dram_tensor(name, shape, fp32, kind="ExternalInput").ap()

    def dinr(name, shape):
        return nc.dram_tensor(name, shape, fp32r, kind="ExternalInput").ap()

    x_d = dinr("x_r", [NT, P, C])
    wq0t_d = dinr("wq0t_r", [NT, P, N])
    wqk1t_d = dinr("wqk1t_r", [NT, P, 1024])
    wv1t_d = dinr("wv1t_r", [NT, P, 512])
    wp0t_d = dinr("wp0t_r", [NT, P, N])
    wp1t_d = dinr("wp1t_r", [4, P, 1024])
    bq0_d = din("bq0_rep", [P, N])
    bqk1_d = din("bqk1_t", [P, 8])
    bv1_d = din("bv1_rep", [P, 512])
    ones_d = dinr("ones_r", [P, 64])
    zeros_d = dinr("zeros_r", [P, 1024])
    out_d = nc.dram_tensor("out_r", [NT, P, C], fp32, kind="ExternalOutput").ap()

    def r(ap):
        return ap.bitcast(fp32r)

    with tile.TileContext(nc) as tc:
      for _rep in range(reps):
        # Pools are opened/closed explicitly; release order must be LIFO.
        rp_cm = tc.tile_pool(name="rp", bufs=3)
        stp_cm = tc.tile_pool(name="stage", bufs=3)
        w0p_cm = tc.tile_pool(name="w0p", bufs=1)
        rp = rp_cm.__enter__()
        stp = stp_cm.__enter__()
        w0p = w0p_cm.__enter__()
        # first k-tile of the phase-B weight, prefetched during phase A so the
        # B chains start without waiting for the A->B WAR-gated reload
        wqk1t0_sb = w0p.tile([P, 1024], fp32r, name="wqk1t0_sb")

        # qkv pool: kT packed (2 heads/tile), q zero-padded per head so the
        # score matmuls are K=128 (uniform tile mode, no PE drains), v with an
        # all-ones block so [Z;U] comes out of one matmul with Z at base 0.
        qkv_cm = tc.tile_pool(name="qkv", bufs=1)
        qkvp = qkv_cm.__enter__()
        kt_sb = qkvp.tile([P, 4, N], fp32r, name="kt_sb")
        qpad_sb = qkvp.tile([P, HPC, N], fp32r, name="qpad_sb")
        vpad_sb = qkvp.tile([P, NT, HPC * VW], fp32r, name="vpad_sb")

        # ---------------- phase A: x1T = Wq0 @ x[b] (transposed out) ----------
        ab_cm = tc.tile_pool(name="ab", bufs=1)
        abp = ab_cm.__enter__()
        wa_cm = tc.tile_pool(name="wa", bufs=1)
        psa_cm = tc.tile_pool(name="psa", bufs=8, space="PSUM")
        wap = wa_cm.__enter__()
        psa = psa_cm.__enter__()
        x_sb = wap.tile([P, NT, C], fp32r, name="x_sb")
        wq0t_sb = wap.tile([P, NT, N], fp32r, name="wq0t_sb")
        for t in range(NT):
            nc.sync.dma_start(out=x_sb[:, t], in_=x_d[t])
            nc.sync.dma_start(out=wq0t_sb[:, t], in_=wq0t_d[t])

        nc.sync.dma_start(out=wqk1t0_sb[:], in_=wqk1t_d[0])
        bq0_sb = wap.tile([P, N], fp32, name="bq0_sb")
        nc.sync.dma_start(out=bq0_sb[:], in_=bq0_d)
        bqk1_sb = abp.tile([P, 8], fp32, name="bqk1_sb")
        nc.sync.dma_start(out=bqk1_sb[:], in_=bqk1_d)
        bv1_sb = abp.tile([P, 512], fp32, name="bv1_sb")
        nc.sync.dma_start(out=bv1_sb[:], in_=bv1_d)
        x1t_sb = abp.tile([P, NT, N], fp32r, name="x1t_sb")
        ones_sb = abp.tile([P, HPC], fp32r, name="ones_sb")
        nc.sync.dma_start(out=ones_sb[:], in_=ones_d[:, 0:HPC])
        for t in range(NT):
            od = vpad_sb[:, t, :].rearrange(
                "p (h j) -> p h j", h=HPC)[:, :, HD:VW]
            nc.vector.tensor_copy(od, ones_sb[:, :, None])
        # zero the unused halves of the padded q tiles on DVE (idle in A)
        zeros_sb = wap.tile([P, N], fp32r, name="zeros_sb")
        nc.sync.dma_start(out=zeros_sb[:], in_=zeros_d)
        for h in range(HPC):
            beta = 64 * ((h + 1) % 2)
            nc.vector.tensor_copy(
                qpad_sb[beta:beta + 64, h, :], zeros_sb[beta:beta + 64, :])

        # activation table warm-up so the ~2.7us exp table load overlaps phase A
        warm = stp.tile([P, 32], fp32, name="warm", bufs=1)
        nc.gpsimd.memset(warm[:], 0.0)
        nc.scalar.activation(warm[0:1, :], warm[0:1, :], Exp)

        for mc in range(2):
            msl = slice(512 * mc, 512 * mc + 512)
            for ct in range(NT):
                ps = psa.tile([P, 512], fp32, tag="psa", name="ps_a")
                for k in range(NT):
                    nc.tensor.matmul(
                        ps[:],
                        x_sb[:, k, 128 * ct:128 * ct + 128],
                        wq0t_sb[:, k, msl],
                        start=(k == 0), stop=(k == NT - 1),
                    )
                nc.vector.tensor_add(x1t_sb[:, ct, msl], ps[:], bq0_sb[:, msl])
        psa_cm.__exit__(None, None, None)
        wa_cm.__exit__(None, None, None)

        # -------- phase B: qkT = Wqk1 @ x1 (transposed), v natural ------------
        wb_cm = tc.tile_pool(name="wb", bufs=1)
        psb_cm = tc.tile_pool(name="psb", bufs=8, space="PSUM")
        wbp = wb_cm.__enter__()
        psb = psb_cm.__enter__()
        wqk1t_sb = wbp.tile([P, NT, 1024], fp32r, name="wqk1t_sb")
        wv1t_sb = wbp.tile([P, NT, 512], fp32r, name="wv1t_sb")
        for t in range(1, NT):
            nc.sync.dma_start(out=wqk1t_sb[:, t], in_=wqk1t_d[t])
        for t in range(NT):
            nc.sync.dma_start(out=wv1t_sb[:, t], in_=wv1t_d[t])

        def bqk_wtile(k, dt):
            if k == 0:
                return wqk1t0_sb[:, 128 * dt:128 * dt + 128]
            return wqk1t_sb[:, k, 128 * dt:128 * dt + 128]

        for dt in range(NT):
            for nch in range(2):
                nsl = slice(512 * nch, 512 * nch + 512)
                ps = psb.tile([P, 512], fp32, tag="psb", name="ps_b")
                for k in range(NT):
                    nc.tensor.matmul(
                        ps[:],
                        bqk_wtile(k, dt),
                        x1t_sb[:, k, nsl],
                        start=(k == 0), stop=(k == NT - 1),
                    )
                if dt < 4:
                    # q tile pair -> zero-padded per-head layout
                    # (one half on DVE, one on ACT so neither paces the chain)
                    nc.vector.tensor_scalar_add(
                        qpad_sb[0:64, 2 * dt, nsl], ps[0:64, :],
                        bqk1_sb[0:64, dt:dt + 1])
                    nc.scalar.activation(
                        qpad_sb[64:P, 2 * dt + 1, nsl], ps[64:P, :], Ident,
                        bias=bqk1_sb[64:P, dt:dt + 1])
                elif dt % 2 == 0:
                    nc.vector.tensor_scalar_add(
                        kt_sb[:, dt - 4, nsl], ps[:], bqk1_sb[:, dt:dt + 1])
                else:
                    nc.scalar.activation(
                        kt_sb[:, dt - 4, nsl], ps[:], Ident,
                        bias=bqk1_sb[:, dt:dt + 1])

        for nt in range(NT):
            ps = psb.tile([P, 512], fp32, tag="psb", name="ps_v")
            for k in range(NT):
                nc.tensor.matmul(
                    ps[:],
                    x1t_sb[:, k, 128 * nt:128 * nt + 128],
                    wv1t_sb[:, k, :],
                    start=(k == 0), stop=(k == NT - 1),
                )
            vdst = vpad_sb[:, nt, :].rearrange(
                "p (h j) -> p h j", h=HPC)[:, :, 0:HD]
            vsrc = ps[:].rearrange("p (h j) -> p h j", h=HPC)
            bsrc = bv1_sb[:].rearrange("p (h j) -> p h j", h=HPC)
            nc.vector.tensor_tensor(vdst, vsrc, bsrc, mybir.AluOpType.add)
        psb_cm.__exit__(None, None, None)
        wb_cm.__exit__(None, None, None)
        ab_cm.__exit__(None, None, None)

        # ------------- phase C: attention, software-pipelined across pairs ----
        otp_cm = tc.tile_pool(name="otp", bufs=1)
        otp = otp_cm.__enter__()
        ot_sb = otp.tile([P, 4, N], fp32r, name="ot_sb")

        wd_cm = tc.tile_pool(name="wd", bufs=1)
        wdp = wd_cm.__enter__()
        # prefetch the big proj seq weight during C
        wp0t_sb = wdp.tile([P, NT, N], fp32r, name="wp0t_sb")
        for t in range(NT):
            nc.sync.dma_start(out=wp0t_sb[:, t], in_=wp0t_d[t])

        psu_cm = tc.tile_pool(name="psu", bufs=4, space="PSUM")
        pst_cm = tc.tile_pool(name="pst", bufs=2, space="PSUM")
        ep_cm = tc.tile_pool(name="ep", bufs=8)
        psu = psu_cm.__enter__()
        pst = pst_cm.__enter__()
        ep = ep_cm.__enter__()

        def u_step(state, k):
            pi, et, ups = state
            for hh in range(2):
                h = 2 * pi + hh
                lhsT = vpad_sb[:, k, VW * h:VW * h + VW]
                for nch in range(2):
                    nsl = slice(512 * nch, 512 * nch + 512)
                    nc.tensor.matmul(
                        ups[(hh, nch)][0:VW, :],
                        lhsT,
                        et[(hh, k // 2)][:, k % 2, nsl],
                        start=(k == 0), stop=(k == NT - 1),
                    )

        def u_tails(state):
            pi, et, ups = state
            for hh in range(2):
                h = 2 * pi + hh
                ct = h // 2
                beta = 64 * (h % 2)
                for nch in range(2):
                    nsl = slice(512 * nch, 512 * nch + 512)
                    u = ups[(hh, nch)]
                    rr = rp.tile([P, 512], fp32, tag="r", name="r_sb")
                    nc.vector.reciprocal(rr[64:65, :], u[64:65, :])
                    # NOTE: the partition_broadcast ucode reads partition 0 of
                    # the tile regardless of the AP base (HW-verified), so hop
                    # the scalar row down to partition 0 with a tiny DMA first.
                    rr0 = rp.tile([P, 512], fp32, tag="r0", name="r0_sb")
                    nc.sync.dma_start(out=rr0[0:1, :], in_=rr[64:65, :])
                    r128 = rp.tile([P, 512], fp32, tag="r128", name="r128_sb")
                    nc.gpsimd.partition_broadcast(r128[0:HD, :], rr0[0:1, :])
                    ostage = stp.tile([P, 512], fp32r, tag="ost", name="ostage")
                    nc.vector.tensor_mul(
                        ostage[0:HD, :], u[0:HD, :], r128[0:HD, :])
                    nc.sync.dma_start(
                        out=ot_sb[beta:beta + 64, ct, nsl],
                        in_=ostage[0:HD, :])

        prev = None
        for pi in range(4):
            et = {}
            for hh in range(2):
                for mh in range(4):
                    et[(hh, mh)] = ep.tile([P, 2, N], fp32r, tag="e", name="e_sb")
            for mt in range(NT):
                for hh in range(2):
                    h = 2 * pi + hh
                    st = pst.tile([P, N], fp32, tag="st", name="ps_st")
                    for nch in range(2):
                        nsl = slice(512 * nch, 512 * nch + 512)
                        nc.tensor.matmul(
                            st[:, nsl],
                            kt_sb[:, h // 2, 128 * mt:128 * mt + 128],
                            qpad_sb[:, h, nsl],
                            start=True, stop=True,
                        )
                    nc.scalar.activation(
                        et[(hh, mt // 2)][:, mt % 2, :], st[:], Exp,
                        scale=SCALE)
                if prev is not None:
                    u_step(prev, mt)
            if prev is not None:
                u_tails(prev)
            ups = {}
            for hh in range(2):
                for nch in range(2):
                    ups[(hh, nch)] = psu.tile([P, 512], fp32, tag="u", name="ps_u")
            prev = (pi, et, ups)
        for k in range(NT):
            u_step(prev, k)
        # free ep/pst now so E' matmuls can run during the final softmax tails
        ep_cm.__exit__(None, None, None)
        pst_cm.__exit__(None, None, None)
        u_tails(prev)

        # ---------- phase E'/D': proj (channel partial, then seq) -------------
        yp_cm = tc.tile_pool(name="yp", bufs=1)
        psd_cm = tc.tile_pool(name="psd", bufs=4, space="PSUM")
        yp = yp_cm.__enter__()
        psd = psd_cm.__enter__()
        y_sb = yp.tile([P, NT, C], fp32r, name="y_sb")
        wp1t_sb = yp.tile([P, 4, 1024], fp32r, name="wp1t_sb")
        for dch in range(2):
            dsl = slice(512 * dch, 512 * dch + 512)
            for t in range(4):
                nc.sync.dma_start(out=wp1t_sb[:, t, dsl], in_=wp1t_d[t, :, dsl])

        for dch in range(2):
            dsl = slice(512 * dch, 512 * dch + 512)
            for nt in range(NT):
                ps = psd.tile([P, 512], fp32, tag="psd", name="ps_y")
                for k in range(4):
                    nc.tensor.matmul(
                        ps[:],
                        ot_sb[:, k, 128 * nt:128 * nt + 128],
                        wp1t_sb[:, k, dsl],
                        start=(k == 0), stop=(k == 3),
                    )
                if nt % 2 == 0:
                    nc.vector.tensor_copy(y_sb[:, nt, dsl], ps[:])
                else:
                    nc.scalar.copy(y_sb[:, nt, dsl], ps[:])
            for mt in range(NT):
                ps = psd.tile([P, 512], fp32, tag="psd", name="ps_o")
                for k in range(NT):
                    nc.tensor.matmul(
                        ps[:],
                        wp0t_sb[:, k, 128 * mt:128 * mt + 128],
                        y_sb[:, k, dsl],
                        start=(k == 0), stop=(k == NT - 1),
                    )
                ostage = stp.tile([P, 512], fp32, tag="ost", name="out_stage")
                if mt % 2 == 0:
                    nc.vector.tensor_copy(ostage[:], ps[:])
                else:
                    nc.scalar.copy(ostage[:], ps[:])
                nc.sync.dma_start(out=out_d[mt, :, dsl], in_=ostage[:])
        psd_cm.__exit__(None, None, None)
        yp_cm.__exit__(None, None, None)
        psu_cm.__exit__(None, None, None)
        wd_cm.__exit__(None, None, None)
        otp_cm.__exit__(None, None, None)
        qkv_cm.__exit__(None, None, None)
        w0p_cm.__exit__(None, None, None)
        stp_cm.__exit__(None, None, None)
        rp_cm.__exit__(None, None, None)

    nc.compile()
    return nc


def _get_nc(reps=1):
    key = ("nc", reps)
    if key not in _CACHE:
        _CACHE[key] = _build(reps)
    return _CACHE[key]


def _in_maps(x, Wq0, bq0, Wq1, bq1, Wp0, bp0, Wp1, bp1):
    f = np.float32
    x = np.asarray(x, f)
    Wq0 = np.asarray(Wq0, f); bq0 = np.asarray(bq0, f)
    Wq1 = np.asarray(Wq1, f); bq1 = np.asarray(bq1, f)
    Wp0 = np.asarray(Wp0, f); Wp1 = np.asarray(Wp1, f)
    wq0t = np.ascontiguousarray(Wq0.T.reshape(NT, P, N))
    wp0t = np.ascontiguousarray(Wp0.T.reshape(NT, P, N))
    bq0r = np.ascontiguousarray(np.broadcast_to(bq0, (P, N)))
    maps = []
    for core in range(8):
        b, g = core // 2, core % 2
        qs = slice(512 * g, 512 * g + 512)
        ks = slice(1024 + 512 * g, 1024 + 512 * g + 512)
        vs = slice(2048 + 512 * g, 2048 + 512 * g + 512)
        wqk1 = np.concatenate([Wq1[qs], Wq1[ks]], 0)          # (1024 d', 1024 c)
        m = {
            "x_r": np.ascontiguousarray(x[b].reshape(NT, P, C)),
            "wq0t_r": wq0t,
            "wqk1t_r": np.ascontiguousarray(wqk1.T.reshape(NT, P, 1024)),
            "wv1t_r": np.ascontiguousarray(Wq1[vs].T.reshape(NT, P, 512)),
            "wp0t_r": wp0t,
            "wp1t_r": np.ascontiguousarray(Wp1[:, qs].T.reshape(4, P, 1024)),
            "bq0_rep": bq0r,
            "bqk1_t": np.ascontiguousarray(
                np.concatenate([bq1[qs], bq1[ks]]).reshape(8, P).T),
            "bv1_rep": np.ascontiguousarray(np.broadcast_to(bq1[vs], (P, 512))),
            "ones_r": np.ones((P, 64), f),
            "zeros_r": np.zeros((P, 1024), f),
        }
        maps.append(m)
    return maps


def kernel(x, Wq0, bq0, Wq1, bq1, Wp0, bp0, Wp1, bp1):
    global LAST_RESULT
    import os

    # The SPMD execute path needs jax's axon PJRT backend; a harness that
    # pinned JAX_PLATFORMS=cpu (common for running the jax reference) would
    # otherwise hide the NeuronCores from this process.
    if "axon" not in os.environ.get("JAX_PLATFORMS", "axon"):
        os.environ.pop("JAX_PLATFORMS", None)
    # This container lacks antenv.axon_hooks, so the BASS_TRACE=1 NTFF path
    # in run_bass_kernel_spmd raises ModuleNotFoundError. Force tracing off
    # (a crash would otherwise replace a working run).
    os.environ["BASS_NEVER_TRACE"] = "1"
    from concourse.bass_utils import run_bass_kernel_spmd

    nc = _get_nc()
    maps = _in_maps(x, Wq0, bq0, Wq1, bq1, Wp0, bp0, Wp1, bp1)
    res = run_bass_kernel_spmd(nc, maps, list(range(8)))
    LAST_RESULT = res
    parts = [np.asarray(r["out_r"]).reshape(N, C) for r in res.results]
    f = np.float32
    bp0 = np.asarray(bp0, f); bp1 = np.asarray(bp1, f)
    Wp1 = np.asarray(Wp1, f)
    bias = np.outer(bp0, Wp1.sum(axis=1)) + bp1[None, :]
    out = np.stack(
        [parts[2 * b] + parts[2 * b + 1] + bias for b in range(B)], 0)
    return out.astype(f)



# revision 34
# speedup vs baseline: 1.2498x; 1.2498x over previous
"""Trainium2 Bass kernel for nn_Attention_56487409877769.

NdLinear-qkv -> 16-head attention -> NdLinear-proj, B=4 N=1024 C=1024 H=16.

Sharding: 8 cores = (batch b, head-group g) with b=core//2, g=core%2.
Each core handles batch b and its 8 heads (qkv channel slice 512g:512g+512).
The proj channel matmul is a partial sum over the core's channel slice; the
host adds the two partials per batch plus a rank-1 bias term (the NdLinear
proj biases commute: out = Wp0 @ O @ Wp1c.T + outer(bp0, Wp1.sum(1)) + bp1).

v3 design notes:
- On-device dtype-SIZE-converting writes (fp32 PSUM -> fp16/fp8 SBUF) are
  broken for downstream PE consumers in this stack (verified by minimal
  repro; DMA readers see such tiles fine).  Therefore every PE-consumed
  tile that is produced on-device is fp32r (same-size as its fp32 source);
  narrow dtypes are used only where the data is HOST-prepared and
  DMA-written (phase-A fp8 DoubleRow inputs) or DMA-consumed (fp16 output
  staging).
- A : x1T[c,m] = sum_n x[n,c] Wq0T[n,m] + bq0[m] with x, Wq0T in fp8e4 and
  DoubleRow matmuls (2 k-tiles per instruction at 0.5 cyc/row): 64 instrs.
- B : qkT = Wqk1 @ x1 (fp32r), q/k packed 2 heads per 128-row tile in
  [d, n] layout; v natural [n, d] in [v|ones]-blocks of 65 per head.
- C : per head: scores.T = kT(64p,128) x q(64p,512) -> PSUM [128, 1024]
  double-buffered; exp on ACT -> E fp32r (the ~66us pacer);
  U = [v|ones].T @ E (N=512, fp32r full rate) -> [65, 1024] with Z in row
  64; PE-transpose per qtile ([65,128] -> [128,65]) puts U AND Z on the
  query partitions, so normalize is one reciprocal + one per-qtile
  multiply on DVE - no partition broadcast, no DMA hop; O lands natural
  [q, d] in fp32r.  T.T = (Wp0 @ O).T chains interleaved per head-pair
  under the exp stream (contraction over q -> per-pair pipelining, which
  the baseline's proj order could not do).
- D : out = T @ Wp1c.T (fp32r), fp16 staging (DMA-only consumer), DMA out.

PSUM in C: scores 2x2 banks + U 2 + transpose 1 + T.T 2 = 8 bank budget.
"""

import sys

if "/opt/trn_rl_repo" not in sys.path:
    sys.path.insert(0, "/opt/trn_rl_repo")

import numpy as np

B, N, C, H = 4, 1024, 1024, 16
HD = C // H          # 64
SCALE = HD ** -0.5
P = 128
NT = N // P          # 8 partition tiles of the 1024 axes
HPC = 8              # heads per core
VW = HD + 1          # [v | ones] block width per head

_CACHE = {}

LAST_RESULT = None   # test.py reads exec_time_ns / profile off this


def _tt_matmuls(nc, ptt, o_sb, wp0t_sb, tt_sb, pi):
    """T.T[d-block pi] = sum_q O[q, d-block] x Wp0.T[q, m] (lhsT=O natural):
    16 fp32r matmuls + 2 copy-outs, returned as thunks so the emitter can
    interleave them a few per exp step under the ACT stream."""
    import concourse.mybir as mybir

    fp32 = mybir.dt.float32

    thunks = []
    box = {}

    def mk(mch, k):
        def f():
            if k == 0:
                box[mch] = ptt.tile([P, 512], fp32, tag="tt", name="ps_tt")
            nc.tensor.matmul(
                box[mch][:],
                o_sb[:, k, 128 * pi:128 * pi + 128],
                wp0t_sb[:, k, 512 * mch:512 * mch + 512],
                start=(k == 0), stop=(k == 7),
            )
        return f

    def mkcopy(mch):
        def f():
            nc.vector.tensor_copy(
                tt_sb[:, pi, 512 * mch:512 * mch + 512], box[mch][:])
        return f

    for mch in range(2):
        for k in range(8):
            thunks.append(mk(mch, k))
        thunks.append(mkcopy(mch))
    return thunks


def _build(reps=1):
    import concourse.mybir as mybir
    import concourse.tile as tile
    from concourse import bacc

    fp32 = mybir.dt.float32
    fp32r = mybir.dt.float32r
    fp8 = mybir.dt.float8e4
    f16 = mybir.dt.float16
    Exp = mybir.ActivationFunctionType.Exp
    Ident = mybir.ActivationFunctionType.Identity
    DR = mybir.MatmulPerfMode.DoubleRow
    Add = mybir.AluOpType.add
    Mult = mybir.AluOpType.mult

    nc = bacc.Bacc("TRN2", target_bir_lowering=False, debug=False)

    def din(name, shape, dt):
        return nc.dram_tensor(name, shape, dt, kind="ExternalInput").ap()

    x_d = din("x8", [NT, P, C], fp8)
    wq0_d = din("wq0t8", [NT, P, N], fp8)
    wqk1_d = din("wqk1t_r", [NT, P, 1024], fp32r)
    wv1_d = din("wv1t_r", [NT, P, 512], fp32r)
    bq0_d = din("bq0_rep", [P, N], fp32)
    bqk1_d = din("bqk1_t", [P, 8], fp32)
    bv1_d = din("bv1_rep", [P, 512], fp32)
    wp0_d = din("wp0t_r", [NT, P, N], fp32r)
    wp1_d = din("wp1t_r", [4, P, C], fp32r)
    id65_d = din("id65_f", [P, 65], fp32)
    ones_d = din("ones_r", [P, 8], fp32r)
    out_d = nc.dram_tensor("out16", [NT, P, C], f16, kind="ExternalOutput").ap()

    with tile.TileContext(nc) as tc:
      for _rep in range(reps):
        # ---------------- pools (LIFO close order) ---------------------------
        stp_cm = tc.tile_pool(name="stage", bufs=4)
        stp = stp_cm.__enter__()
        rp_cm = tc.tile_pool(name="rp", bufs=2)
        rp = rp_cm.__enter__()

        qkv_cm = tc.tile_pool(name="qkv", bufs=1)
        qkvp = qkv_cm.__enter__()
        # q/k: [128 part = 2 heads x 64 d, head-pair, 1024 n]
        q_sb = qkvp.tile([P, 4, N], fp32r, name="q_sb")
        kt_sb = qkvp.tile([P, 4, N], fp32r, name="kt_sb")
        vpad_sb = qkvp.tile([P, NT, HPC * VW], fp32r, name="vpad_sb")
        id65_sb = qkvp.tile([P, 65], fp32, name="id65_sb")
        nc.sync.dma_start(out=id65_sb[:], in_=id65_d)
        ones_sb = qkvp.tile([P, HPC], fp32r, name="ones_sb")
        nc.sync.dma_start(out=ones_sb[:], in_=ones_d)
        for t in range(NT):
            od = vpad_sb[:, t, :].rearrange(
                "p (h j) -> p h j", h=HPC)[:, :, HD:VW]
            nc.vector.tensor_copy(od, ones_sb[:, :, None])

        # ---------------- phase A: x1T = Wq0 @ x[b] (fp8 DoubleRow) -----------
        ab_cm = tc.tile_pool(name="ab", bufs=1)
        abp = ab_cm.__enter__()
        wb_cm = tc.tile_pool(name="wb", bufs=1)
        wbp = wb_cm.__enter__()
        wa_cm = tc.tile_pool(name="wa", bufs=1)
        wap = wa_cm.__enter__()
        psa_cm = tc.tile_pool(name="psa", bufs=4, space="PSUM")
        psa = psa_cm.__enter__()

        x_sb = wap.tile([P, NT, C], fp8, name="x_sb")
        wq0t_sb = wap.tile([P, NT, N], fp8, name="wq0t_sb")
        bq0_sb = wap.tile([P, N], fp32, name="bq0_sb")
        x1t_sb = abp.tile([P, NT, N], fp32r, name="x1t_sb")
        wqk1t_sb = wbp.tile([P, NT, 1024], fp32r, name="wqk1t_sb")
        wv1t_sb = wbp.tile([P, NT, 512], fp32r, name="wv1t_sb")
        bqk1_sb = wbp.tile([P, 8], fp32, name="bqk1_sb")
        bv1_sb = wbp.tile([P, 512], fp32, name="bv1_sb")

        # interleave x/wq0t pair-DMAs so the chains can start early
        nc.sync.dma_start(out=x_sb[:, 0:2], in_=x_d[0:2])
        nc.sync.dma_start(out=wq0t_sb[:, 0:2], in_=wq0_d[0:2])
        nc.sync.dma_start(out=bq0_sb[:], in_=bq0_d)
        for t in range(1, 4):
            nc.sync.dma_start(out=x_sb[:, 2 * t:2 * t + 2], in_=x_d[2 * t:2 * t + 2])
            nc.sync.dma_start(
                out=wq0t_sb[:, 2 * t:2 * t + 2], in_=wq0_d[2 * t:2 * t + 2])
        for t in range(NT):
            nc.sync.dma_start(out=wqk1t_sb[:, t], in_=wqk1_d[t])
        nc.sync.dma_start(out=bqk1_sb[:], in_=bqk1_d)
        for t in range(NT):
            nc.sync.dma_start(out=wv1t_sb[:, t], in_=wv1_d[t])
        nc.sync.dma_start(out=bv1_sb[:], in_=bv1_d)

        # A chains: groups of 4 (2 ct x 2 mch), j-outer DoubleRow over k-pairs
        for cg in range(4):
            cts = (2 * cg, 2 * cg + 1)
            ps = {}
            for ct in cts:
                for mch in range(2):
                    ps[(ct, mch)] = psa.tile([P, 512], fp32, tag="psa", name="ps_a")
            for j in range(4):
                for ct in cts:
                    for mch in range(2):
                        nc.tensor.matmul(
                            ps[(ct, mch)][:],
                            x_sb[:, 2 * j:2 * j + 2, 128 * ct:128 * ct + 128],
                            wq0t_sb[:, 2 * j:2 * j + 2, 512 * mch:512 * mch + 512],
                            start=(j == 0), stop=(j == 3),
                            perf_mode=DR,
                        )
            for ct in cts:
                for mch in range(2):
                    msl = slice(512 * mch, 512 * mch + 512)
                    nc.vector.tensor_tensor(
                        x1t_sb[:, ct, msl], ps[(ct, mch)][:], bq0_sb[:, msl], Add)
        psa_cm.__exit__(None, None, None)
        wa_cm.__exit__(None, None, None)

        # -------- phase B: q,k (score layout) and v (natural, fp32r) ----------
        psb_cm = tc.tile_pool(name="psb", bufs=4, space="PSUM")
        psb = psb_cm.__enter__()

        for gi, dts in enumerate(((0, 1), (4, 5), (2, 3), (6, 7))):
            ps = {}
            for dt in dts:
                for mch in range(2):
                    ps[(dt, mch)] = psb.tile([P, 512], fp32, tag="psb", name="ps_b")
            for j in range(NT):
                for dt in dts:
                    for mch in range(2):
                        nc.tensor.matmul(
                            ps[(dt, mch)][:],
                            wqk1t_sb[:, j, 128 * dt:128 * dt + 128],
                            x1t_sb[:, j, 512 * mch:512 * mch + 512],
                            start=(j == 0), stop=(j == NT - 1),
                        )
            for dt in dts:
                for mch in range(2):
                    msl = slice(512 * mch, 512 * mch + 512)
                    dst = q_sb if dt < 4 else kt_sb
                    if gi < 2:
                        nc.vector.tensor_scalar_add(
                            dst[:, dt % 4, msl], ps[(dt, mch)][:],
                            bqk1_sb[:, dt:dt + 1])
                    else:
                        # later head-pair tiles: ACT is still idle here
                        nc.scalar.activation(
                            dst[:, dt % 4, msl], ps[(dt, mch)][:], Ident,
                            bias=bqk1_sb[:, dt:dt + 1])

        for ng in range(2):
            nts = range(4 * ng, 4 * ng + 4)
            ps = {}
            for nt in nts:
                ps[nt] = psb.tile([P, 512], fp32, tag="psb", name="ps_v")
            for j in range(NT):
                for nt in nts:
                    nc.tensor.matmul(
                        ps[nt][:],
                        x1t_sb[:, j, 128 * nt:128 * nt + 128],
                        wv1t_sb[:, j, :],
                        start=(j == 0), stop=(j == NT - 1),
                    )
            for nt in nts:
                vdst = vpad_sb[:, nt, :].rearrange(
                    "p (h j) -> p h j", h=HPC)[:, :, 0:HD]
                vsrc = ps[nt][:].rearrange("p (h j) -> p h j", h=HPC)
                bsrc = bv1_sb[:].rearrange("p (h j) -> p h j", h=HPC)
                nc.vector.tensor_tensor(vdst, vsrc, bsrc, Add)
        psb_cm.__exit__(None, None, None)
        wb_cm.__exit__(None, None, None)
        ab_cm.__exit__(None, None, None)

        # ------------- phase C: attention, pipelined per head -----------------
        otp_cm = tc.tile_pool(name="otp", bufs=1)
        otp = otp_cm.__enter__()
        o_sb = otp.tile([P, NT, 512], fp32r, name="o_sb")
        tt_sb = otp.tile([P, 4, N], fp32r, name="tt_sb")
        ut_sb = otp.tile([P, N], fp32, name="ut_sb")

        wd_cm = tc.tile_pool(name="wd", bufs=1)
        wdp = wd_cm.__enter__()
        wp0t_sb = wdp.tile([P, NT, N], fp32r, name="wp0t_sb")
        for t in range(NT):
            nc.sync.dma_start(out=wp0t_sb[:, t], in_=wp0_d[t])

        pst_cm = tc.tile_pool(name="pst", bufs=2, space="PSUM")
        psu_cm = tc.tile_pool(name="psu", bufs=1, space="PSUM")
        ptr_cm = tc.tile_pool(name="ptr", bufs=1, space="PSUM")
        ptt_cm = tc.tile_pool(name="ptt", bufs=1, space="PSUM")
        ep_cm = tc.tile_pool(name="ep", bufs=16)
        pst = pst_cm.__enter__()
        psu = psu_cm.__enter__()
        ptr = ptr_cm.__enter__()
        ptt = ptt_cm.__enter__()
        ep = ep_cm.__enter__()

        def u_steps(state, j):
            # U chain steps for key-tiles (2j, 2j+1) of the previous head:
            # ups[65, 1024] += [v|ones].T @ E   (rows 0..63 = U.T, row 64 = Z)
            h, et, ups = state
            vsl = slice(VW * h, VW * h + VW)
            for k in (2 * j, 2 * j + 1):
                for nch in range(2):
                    nc.tensor.matmul(
                        ups[0:VW, 512 * nch:512 * nch + 512],
                        vpad_sb[:, k, vsl],
                        et[k][:, 512 * nch:512 * nch + 512],
                        start=(k == 0), stop=(k == NT - 1),
                    )

        def u_finish(state):
            # previous head's U: PSUM [65, 1024] -> SBUF, then per qtile a
            # PE transpose puts [q,(U.T|Z)] on query partitions; normalize
            # with a reciprocal + per-qtile multiply, landing natural-layout
            # fp32r O.
            h, et, ups = state
            nc.vector.tensor_copy(ut_sb[0:VW, :], ups[0:VW, :])
            rz = rp.tile([P, 8], fp32, tag="rz", name="rz_sb")
            for qt in range(NT):
                tr = ptr.tile([P, VW], fp32, tag="tr", name="ps_tr")
                nc.tensor.matmul(
                    tr[:], ut_sb[0:VW, 128 * qt:128 * qt + 128], id65_sb[0:VW, :],
                    start=True, stop=True, is_transpose=True,
                )
                nc.vector.reciprocal(rz[:, qt:qt + 1], tr[:, HD:VW])
                nc.vector.tensor_scalar_mul(
                    o_sb[:, qt, 64 * h:64 * h + 64], tr[:, 0:HD],
                    rz[:, qt:qt + 1])

        prev = None
        ttq = []  # pending proj-T.T thunks, drained a few per exp step
        for h in range(HPC):
            tp, a = h // 2, h % 2
            psl = slice(64 * a, 64 * a + 64)
            et = [None] * NT
            for j in range(4):
                if prev is not None:
                    u_steps(prev, j)
                for i in range(2):
                    mt = 2 * j + i
                    # [128,1024] score tiles, bufs=2: the next tile's scores
                    # run while ACT exps the previous one (ACT stays saturated)
                    ps = pst.tile([P, 1024], fp32, tag="st", name="ps_st")
                    for mch in range(2):
                        nc.tensor.matmul(
                            ps[:, 512 * mch:512 * mch + 512],
                            kt_sb[psl, tp, 128 * mt:128 * mt + 128],
                            q_sb[psl, tp, 512 * mch:512 * mch + 512],
                            start=True, stop=True,
                        )
                    etj = ep.tile([P, 1024], fp32r, tag="e", name="e_sb")
                    nc.scalar.activation(etj[:], ps[:], Exp, scale=SCALE)
                    et[mt] = etj
                    for _ in range(min(2, len(ttq))):
                        ttq.pop(0)()
            if prev is not None:
                u_finish(prev)
                if prev[0] % 2 == 1:
                    ttq += _tt_matmuls(nc, ptt, o_sb, wp0t_sb, tt_sb, prev[0] // 2)
            ups = psu.tile([P, 1024], fp32, tag="u", name="ps_u")
            prev = (h, et, ups)
        for j in range(4):
            u_steps(prev, j)
            for _ in range(min(4, len(ttq))):
                ttq.pop(0)()
        u_finish(prev)
        ttq += _tt_matmuls(nc, ptt, o_sb, wp0t_sb, tt_sb, 3)
        for fn in ttq:
            fn()
        ep_cm.__exit__(None, None, None)
        ptt_cm.__exit__(None, None, None)
        ptr_cm.__exit__(None, None, None)
        psu_cm.__exit__(None, None, None)
        pst_cm.__exit__(None, None, None)

        # ---------- phase D: out = T @ Wp1c.T (fp32r) -------------------------
        wd2_cm = tc.tile_pool(name="wd2", bufs=1)
        wd2p = wd2_cm.__enter__()
        wp1t_sb = wd2p.tile([P, 4, C], fp32r, name="wp1t_sb")
        for t in range(4):
            nc.sync.dma_start(out=wp1t_sb[:, t], in_=wp1_d[t])
        psd_cm = tc.tile_pool(name="psd", bufs=4, space="PSUM")
        psd = psd_cm.__enter__()
        for mt in range(NT):
            for dch in range(2):
                dsl = slice(512 * dch, 512 * dch + 512)
                ps = psd.tile([P, 512], fp32, tag="psd", name="ps_o")
                for kd in range(4):
                    nc.tensor.matmul(
                        ps[:],
                        tt_sb[:, kd, 128 * mt:128 * mt + 128],
                        wp1t_sb[:, kd, dsl],
                        start=(kd == 0), stop=(kd == 3),
                    )
                ostage = stp.tile([P, 512], f16, tag="ost", name="out_stage")
                if mt % 2 == 0:
                    nc.vector.tensor_copy(ostage[:], ps[:])
                else:
                    nc.scalar.copy(ostage[:], ps[:])
                nc.sync.dma_start(out=out_d[mt, :, dsl], in_=ostage[:])
        psd_cm.__exit__(None, None, None)
        wd2_cm.__exit__(None, None, None)
        wd_cm.__exit__(None, None, None)
        otp_cm.__exit__(None, None, None)
        qkv_cm.__exit__(None, None, None)
        rp_cm.__exit__(None, None, None)
        stp_cm.__exit__(None, None, None)

    nc.compile()
    return nc


def _get_nc(reps=1):
    key = ("nc", reps)
    if key not in _CACHE:
        _CACHE[key] = _build(reps)
    return _CACHE[key]


def _in_maps(x, Wq0, bq0, Wq1, bq1, Wp0, bp0, Wp1, bp1):
    import ml_dtypes

    f = np.float32
    e4 = ml_dtypes.float8_e4m3
    x = np.asarray(x, f)
    Wq0 = np.asarray(Wq0, f); bq0 = np.asarray(bq0, f)
    Wq1 = np.asarray(Wq1, f); bq1 = np.asarray(bq1, f)
    Wp0 = np.asarray(Wp0, f); Wp1 = np.asarray(Wp1, f)
    wq0t8 = np.ascontiguousarray(Wq0.T.reshape(NT, P, N)).astype(e4)
    wp0t = np.ascontiguousarray(Wp0.T.reshape(NT, P, N))
    bq0r = np.ascontiguousarray(np.broadcast_to(bq0, (P, N)))
    id65 = np.zeros((P, 65), f)
    id65[:65, :] = np.eye(65, dtype=f)
    maps = []
    for core in range(8):
        b, g = core // 2, core % 2
        # natural layout: qk tile dt<4 = q head-pair (2dt, 2dt+1), dt>=4 = k
        perm = np.concatenate([
            np.arange(512 * g, 512 * g + 512),
            np.arange(C + 512 * g, C + 512 * g + 512)])
        wqk1 = Wq1[perm]                                      # (1024 d', 1024 c)
        vs = slice(2 * C + 512 * g, 2 * C + 512 * g + 512)
        m = {
            "x8": np.ascontiguousarray(x[b].reshape(NT, P, C)).astype(e4),
            "wq0t8": wq0t8,
            "wqk1t_r": np.ascontiguousarray(wqk1.T.reshape(NT, P, 1024)),
            "wv1t_r": np.ascontiguousarray(Wq1[vs].T.reshape(NT, P, 512)),
            "bq0_rep": bq0r,
            "bqk1_t": np.ascontiguousarray(bq1[perm].reshape(8, P).T),
            "bv1_rep": np.ascontiguousarray(np.broadcast_to(bq1[vs], (P, 512))),
            "wp0t_r": wp0t,
            "wp1t_r": np.ascontiguousarray(
                Wp1[:, 512 * g:512 * g + 512].T.reshape(4, P, C)),
            "id65_f": id65,
            "ones_r": np.ones((P, 8), f),
        }
        maps.append(m)
    return maps


def kernel(x, Wq0, bq0, Wq1, bq1, Wp0, bp0, Wp1, bp1):
    global LAST_RESULT
    import os

    # The SPMD execute path needs jax's axon PJRT backend; a harness that
    # pinned JAX_PLATFORMS=cpu (common for running the jax reference) would
    # otherwise hide the NeuronCores from this process.
    if "axon" not in os.environ.get("JAX_PLATFORMS", "axon"):
        os.environ.pop("JAX_PLATFORMS", None)
    # This container lacks antenv.axon_hooks, so the BASS_TRACE=1 NTFF path
    # in run_bass_kernel_spmd raises ModuleNotFoundError. Force tracing off
    # (a crash would otherwise replace a working run).
    os.environ["BASS_NEVER_TRACE"] = "1"
    from concourse.bass_utils import run_bass_kernel_spmd

    nc = _get_nc()
    maps = _in_maps(x, Wq0, bq0, Wq1, bq1, Wp0, bp0, Wp1, bp1)
    res = run_bass_kernel_spmd(nc, maps, list(range(8)))
    LAST_RESULT = res
    parts = [np.asarray(r["out16"], np.float32).reshape(N, C)
             for r in res.results]
    f = np.float32
    bp0 = np.asarray(bp0, f); bp1 = np.asarray(bp1, f)
    Wp1 = np.asarray(Wp1, f)
    bias = np.outer(bp0, Wp1.sum(axis=1)) + bp1[None, :]
    out = np.stack(
        [parts[2 * b] + parts[2 * b + 1] + bias for b in range(B)], 0)
    return out.astype(f)
